# revision 1
# baseline (speedup 1.0000x reference)
"""Contrastive queue loss kernel for 8 Trainium2 NeuronCores.

Reference computation (all fp32):
    pos[j,b]    = V[j,b,:] . L[b,:] / T                  (J=2, B=256, F=128)
    qlog[j,b,q] = V[j,b,:] . queue[q,:] / T              (Q=65536)
    denom[j,b]  = log( sum_i exp(pos[j,i]) + sum_q exp(qlog[j,b,q]) )
    loss        = -sum_{j,b} (pos[j,b] - denom[j,b]) / B

Sharding: queue split along Q across 8 cores (8192 rows each); V replicated.
pos is computed on the host (it is tiny); each core returns per-jb partial
sums of exp(10*logit) over its queue shard, split over several columns.

Per-core dataflow (three-engine exp; GPSIMD cannot touch PSUM, and
tensor_tensor_reduce is avoided because it wedges the device):
  queue chunk --DMA--> SBUF f32 --Pool cast--> bf16
    --PE identity-transpose--> PSUM --DVE copy--> qts [f, q] bf16
  logits: PE matmul v2T x qts-slices -> PSUM f32
    ringA 2x[128,1536] (6 banks) for ACT, ringB 2x[128,512] (2) for D
  exp+sum split per 512-col unit (see PATS):
    A: ACT exp(10x) in-place + fused accum col            (~70% of cols)
    D: DVE bit-trick tensor_scalar PSUM->int16 SBUF; Pool pre-adds
       bitcast-bf16 halves (tensor_tensor), DVE tensor_reduce emits one
       partial column per 2048-col block
Host combines: pos einsum + log over the summed partial columns.

Bit-trick: exp(x) ~= bitcast_bf16(int16(A*x + B)), A = 128/ln2 scaled by
1/T, B = 127*128 - 7.37 calibrated for zero mean ratio error vs exact exp
(+-4% sawtooth per element cancels to <0.1% in 2048-term sums; the loss
tolerance is 2e-2 and the log damps errors further).
"""

import numpy as np

J, B, F, Q = 2, 256, 128, 65536
NCORES = 8
QC = Q // NCORES          # 8192 queue rows per core
JB = J * B                # 512
INV_T = 10.0
NT = JB // 128            # 4 jb tiles of 128

# Schraudolph constants (bf16 bitcast)
BT_A = INV_T * 128.0 / np.log(2.0)       # 1846.6466
BT_B = 127.0 * 128.0 - 7.37              # 16248.63
RED_SCALE = 1.0                          # host-side correction knob

# Queue chunks (rows). Two small ones first for a fast ramp, then 1536s
# so the ACT ring tiles (and ops) reach 1536 columns.
CHUNKS = [(0, 512), (512, 512), (1024, 1536), (2560, 1536), (4096, 1536),
          (5632, 1536), (7168, 1024)]
N_WARMUP = 16             # dummy PE matmuls to lift the HAM clock gate

# Per-span slot pattern, rotated onto jb tiles by span index.
import os as _os
PATS = (_os.environ.get("KERNEL_PATS") or
        "AAAD,AADD,AAAD,AADD,AAAD,AADD,AAAA").split(",")


def _schedule():
    sched = []
    for si in range(len(CHUNKS)):
        pat = PATS[si]
        sched.append({t: pat[(t + si) % 4] for t in range(NT)})
    return sched


SCHED = _schedule()
N_ACC = sum(1 for s in SCHED for t in s if s[t] == "A")
ACC_MAP = [t for si, s in enumerate(SCHED) for t in range(NT) if s[t] == "A"]
STAG_SZ = [sum(CHUNKS[si][1] for si, s in enumerate(SCHED) if s[t] != "A")
           for t in range(NT)]
STAG_OFF = [sum(STAG_SZ[:t]) for t in range(NT)]
STAG_TOT = sum(STAG_SZ)
# pre-added bit-trick blocks (bf16, half the stag width) are DMAd to the
# host, which does the final summation; BLOCKS = (tile, preg_off, width)
BLOCKS = []
for _t in range(NT):
    _d0 = 0
    while _d0 < STAG_SZ[_t]:
        _take = 2048 if STAG_SZ[_t] - _d0 >= 2048 else STAG_SZ[_t] - _d0
        BLOCKS.append((_t, (STAG_OFF[_t] + _d0) // 2, _take // 2))
        _d0 += _take
NOUT = N_ACC

_STATE = {}


def _build():
    import concourse.tile as tile
    from concourse import bacc, masks, mybir

    f32 = mybir.dt.float32
    bf16 = mybir.dt.bfloat16
    i16 = mybir.dt.int16
    Exp = mybir.ActivationFunctionType.Exp
    mul, add = mybir.AluOpType.mult, mybir.AluOpType.add

    nc = bacc.Bacc("TRN2", target_bir_lowering=False, debug=False,
                   num_devices=NCORES)

    v2_d = nc.dram_tensor("V2", (JB, F), f32, kind="ExternalInput")
    q_d = nc.dram_tensor("queue", (QC, F), f32, kind="ExternalInput")
    out_d = nc.dram_tensor("out", (128, NOUT), f32, kind="ExternalOutput")
    out2_d = nc.dram_tensor("out2", (128, STAG_TOT // 2), bf16,
                            kind="ExternalOutput")

    with tile.TileContext(nc) as tc:
        with (
            tc.tile_pool(name="const", bufs=1) as const_pool,
            tc.tile_pool(name="vl", bufs=1) as vl_pool,
            tc.tile_pool(name="qt", bufs=len(CHUNKS)) as qt_pool,
            tc.tile_pool(name="qtb", bufs=3) as qtb_pool,
            tc.tile_pool(name="big", bufs=1) as big_pool,
            tc.tile_pool(name="res", bufs=1) as res_pool,
            tc.tile_pool(name="ringa", bufs=2, space="PSUM") as ringa_pool,
            tc.tile_pool(name="ringb", bufs=2, space="PSUM") as ringb_pool,
        ):
            # PE clock warmup
            wsrc = const_pool.tile([128, 128], bf16, tag="wsrc")
            nc.vector.memset(wsrc[:], 0.0)
            for _ in range(N_WARMUP):
                lgw = ringa_pool.tile([128, 1536], f32, tag="ringa")
                nc.tensor.matmul(lgw[:, :128], lhsT=wsrc[:], rhs=wsrc[:],
                                 start=True, stop=True)

            # pre-load the ACT Exp table while DMAs are in flight
            twarm = const_pool.tile([128, 8], f32, tag="twarm")
            nc.vector.memset(twarm[:], 0.0)
            nc.scalar.activation(twarm[:], twarm[:], Exp, scale=1.0)

            identb = const_pool.tile([128, 128], bf16, tag="identb")
            masks.make_identity(nc, identb[:])

            # V2T [f=128, jb=512] bf16
            vt_all = vl_pool.tile([128, JB], f32)      # [p, (t f)] natural V2
            nc.sync.dma_start(
                vt_all[:].rearrange("p (t f) -> p t f", f=F),
                v2_d.ap().rearrange("(t p) f -> p t f", p=128))
            vtb = vl_pool.tile([128, JB], bf16)
            nc.vector.tensor_copy(vtb[:], vt_all[:])
            pv = ringa_pool.tile([128, JB], bf16, tag="ringa")
            for t in range(NT):
                nc.tensor.transpose(
                    pv[:, t * 128:(t + 1) * 128],
                    vtb[:, t * 128:(t + 1) * 128], identb[:])
            v2tb = vl_pool.tile([128, JB], bf16)       # [f, jb]
            nc.vector.tensor_copy(v2tb[:], pv[:])

            # persistent transposed queue [f=128, q=8192] bf16
            qts = big_pool.tile([128, QC], bf16)
            # bit-trick staging: int16 results + bf16 logits for the W path
            stag = big_pool.tile([128, STAG_TOT], i16)
            preg = big_pool.tile([128, STAG_TOT // 2], bf16)
            junk = big_pool.tile([128, 2048], bf16)
            acc = res_pool.tile([128, N_ACC], f32)

            cur = [0] * NT            # staging cursor per jb tile
            done = [0] * NT           # cols already pair-summed
            acc_i = [0]

            def flush_red(t, tail=False):
                # DVE pre-adds adjacent halves (int16 bitcast as bf16); the
                # pre-added block goes to the host over the idle DMA track,
                # which does the final summation.  (tensor_tensor_reduce is
                # avoided on-device: it kills the device.)
                while True:
                    avail = cur[t] - done[t]
                    take = 2048 if avail >= 2048 else (avail if tail else 0)
                    if take == 0 or take % 2:
                        break
                    h = take // 2
                    lo = STAG_OFF[t] + done[t]
                    plo = (STAG_OFF[t] + done[t]) // 2
                    a0 = stag[:, lo:lo + h].bitcast(mybir.dt.bfloat16)
                    a1 = stag[:, lo + h:lo + take].bitcast(mybir.dt.bfloat16)
                    nc.gpsimd.tensor_tensor(preg[:, plo:plo + h], a0, a1, add)
                    nc.sync.dma_start(out2_d.ap()[:, plo:plo + h],
                                      preg[:, plo:plo + h])
                    done[t] += take

            def consume_bd(eng, t, lg, w):
                # one ringB unit (w == 512). D: DVE bit-trick ts straight
                # from PSUM. P: DMA the logits to SBUF over the idle DMA
                # track (SP queue), then the Pool engine bit-tricks them
                # (GPSIMD cannot read PSUM directly).
                lo = STAG_OFF[t] + cur[t]
                nc.vector.tensor_scalar(stag[:, lo:lo + w], lg[:, :w],
                                        BT_A, BT_B, mul, add)
                cur[t] += w
                flush_red(t, tail=(cur[t] == STAG_SZ[t]))

            def load(ci):
                r0, nr = CHUNKS[ci]
                qt = qt_pool.tile([128, nr], f32, tag="qt")
                nc.sync.dma_start(
                    qt[:].rearrange("p (s f) -> p s f", f=F),
                    q_d.ap()[r0:r0 + nr, :].rearrange(
                        "(s p) f -> p s f", p=128))
                return qt

            def cast(ci, qt):
                r0, nr = CHUNKS[ci]
                qtb = qtb_pool.tile([128, nr], bf16, tag="qtb")
                eng = nc.vector if ci < 3 else nc.gpsimd
                eng.tensor_copy(qtb[:], qt[:])
                return qtb

            def prep(ci, qtb):
                r0, nr = CHUNKS[ci]
                # PE transpose in <=768-col pieces staged through ringB
                off = 0
                while off < nr:
                    pw = min(768, nr - off)
                    pt = ringb_pool.tile([128, pw], bf16, tag="ringb")
                    for s in range(pw // 128):
                        nc.tensor.transpose(
                            pt[:, s * 128:(s + 1) * 128],
                            qtb[:, off + s * 128:off + (s + 1) * 128],
                            identb[:])
                    nc.vector.tensor_copy(qts[:, r0 + off:r0 + off + pw],
                                          pt[:])
                    off += pw

            def spans(ci, only=None):
                r0, nr = CHUNKS[ci]
                order = sorted(range(NT),
                               key=lambda t: {"A": 0, "D": 1, "P": 2}[
                                   SCHED[ci][t]])
                for t in order:
                    eng = SCHED[ci][t]
                    if only is not None and eng != only:
                        continue
                    lhsT = v2tb[:, t * 128:(t + 1) * 128]
                    if eng == "A":
                        lg = ringa_pool.tile([128, nr], f32, tag="ringa")
                        for k in range(nr // 512):
                            nc.tensor.matmul(
                                lg[:, k * 512:(k + 1) * 512], lhsT=lhsT,
                                rhs=qts[:, r0 + k * 512:r0 + (k + 1) * 512],
                                start=True, stop=True)
                        nc.scalar.activation(
                            lg[:], lg[:], Exp, scale=INV_T,
                            accum_out=acc[:, acc_i[0]:acc_i[0] + 1])
                        acc_i[0] += 1
                    else:
                        for k in range(nr // 512):
                            lg = ringb_pool.tile([128, 512], f32, tag="ringb")
                            nc.tensor.matmul(
                                lg[:], lhsT=lhsT,
                                rhs=qts[:, r0 + k * 512:r0 + (k + 1) * 512],
                                start=True, stop=True)
                            consume_bd(eng, t, lg, 512)

            NCH = len(CHUNKS)
            PIPE_LAG = 2
            N_PRELOAD = 2
            qts_tiles = [load(ci) for ci in range(N_PRELOAD)]
            for ci in range(NCH + PIPE_LAG):
                if N_PRELOAD + ci < NCH:
                    qts_tiles.append(load(N_PRELOAD + ci))
                qtb = cast(ci, qts_tiles[ci]) if ci < NCH else None
                if ci >= PIPE_LAG:
                    spans(ci - PIPE_LAG)
                if ci < NCH:
                    prep(ci, qtb)

            for t in range(NT):
                flush_red(t, tail=True)
                assert done[t] == cur[t], (t, done[t], cur[t])

            nc.sync.dma_start(out_d.ap(), acc[:])

    nc.compile()
    return nc


def _run(in_maps, trace=False, **kwargs):
    from concourse.bass_utils import run_bass_kernel_spmd
    if "nc" not in _STATE:
        _STATE["nc"] = _build()
    return run_bass_kernel_spmd(_STATE["nc"], in_maps, list(range(NCORES)),
                                trace=trace, **kwargs)


def _make_in_maps(V, L, queue):
    V2 = np.ascontiguousarray(
        np.asarray(V, dtype=np.float32).reshape(JB, F))
    qn = np.asarray(queue, dtype=np.float32).reshape(NCORES, QC, F)
    return [{"V2": V2, "queue": np.ascontiguousarray(qn[i])}
            for i in range(NCORES)]


def _combine(V, L, outs):
    """outs: list of ((128, N_ACC), (128, STAG_TOT//2)) per core."""
    qsum = np.zeros((NT, 128), dtype=np.float64)
    for o, o2 in outs:
        o = o.astype(np.float64)
        for c, t in enumerate(ACC_MAP):
            qsum[t] += o[:, c]
        o2 = o2.astype(np.float64)
        for t, plo, h in BLOCKS:
            qsum[t] += RED_SCALE * o2[:, plo:plo + h].sum(axis=1)
    qsum = qsum.reshape(JB)                                  # jb = t*128 + p
    pos_s = INV_T * np.einsum(
        "jbf,bf->jb",
        np.asarray(V, dtype=np.float64),
        np.asarray(L, dtype=np.float64)).reshape(JB)
    batch_sum = np.exp(pos_s).reshape(J, B).sum(axis=1)      # sum_i exp(pos)
    denom = np.log(np.repeat(batch_sum, B) + qsum)
    loss = -(pos_s.sum() - denom.sum()) / B
    return np.array(loss, dtype=np.float32)


def kernel(V, L, queue):
    res = _run(_make_in_maps(V, L, queue))
    return _combine(V, L, [(res.results[i]["out"], res.results[i]["out2"])
                           for i in range(NCORES)])



# revision 7
# speedup vs baseline: 1.1389x; 1.1389x over previous
"""Contrastive queue loss kernel for 8 Trainium2 NeuronCores.

Reference computation (all fp32):
    pos[j,b]    = V[j,b,:] . L[b,:] / T                  (J=2, B=256, F=128)
    qlog[j,b,q] = V[j,b,:] . queue[q,:] / T              (Q=65536)
    denom[j,b]  = log( sum_i exp(pos[j,i]) + sum_q exp(qlog[j,b,q]) )
    loss        = -sum_{j,b} (pos[j,b] - denom[j,b]) / B

Sharding: queue split along Q across 8 cores (8192 rows each); V replicated.
pos is computed on the host (it is tiny); each core returns per-jb partial
sums of exp(10*logit) over its queue shard.

v2 design (vs v1): the queue shard and V2 are sent HOST-TRANSPOSED
([feature, item] layout, same f32 bytes) so the kernel needs no on-chip
transpose, no PSUM->SBUF copy and no bf16 cast.  Matmuls run in float32r
(TF32-like, 1 col/cycle at full clock) straight off the DMAd data.
Logit consumption out of PSUM is split between the only two engines that
can read PSUM (GPSIMD/Pool cannot - BIR verifier rejects it):
  A steps: ACT exp(10x) in-place + fused accum column        (~56% of cols)
  D steps: DVE bit-trick tensor_scalar PSUM->int16 SBUF, Pool pre-adds
           bitcast-bf16 halves (tensor_tensor) into a staging tile that is
           flushed to the host in a few bf16 DMAs; host does the final sum
PSUM is three fixed regions [2048, 1024, 1024] used round-robin, one
consumer op per region-step: consumer i, consumer i+1 and the fill of
step i+2 always touch disjoint regions, so ACT/DVE/PE never WAR-stall.
Steps map to jb tiles round-robin (t = s % 4); lcm(3,4) makes every tile
get exactly 8192 columns.

Bit-trick: exp(x) ~= bitcast_bf16(int16(A*x + B)), A = 128/ln2 scaled by
1/T, B = 127*128 - 7.37 calibrated for zero mean ratio error vs exact exp
(+-4% sawtooth per element cancels to <0.1% in 1000+-term sums; the loss
tolerance is 2e-2 and the log damps errors further).
"""

import numpy as np

J, B, F, Q = 2, 256, 128, 65536
NCORES = 8
QC = Q // NCORES          # 8192 queue rows per core
JB = J * B                # 512
INV_T = 10.0
NT = JB // 128            # 4 jb tiles of 128

# Schraudolph constants (bf16 bitcast)
BT_A = INV_T * 128.0 / np.log(2.0)       # 1846.6466
BT_B = 127.0 * 128.0 - 7.37              # 16248.63

# qT DMA chunk widths (columns = queue rows); small first for a fast ramp
DMA_CHUNKS = [512, 512, 1024, 1536, 1536, 1536, 1536]
assert sum(DMA_CHUNKS) == QC

# PSUM: four 1024-col regions cycled per step; one consumer op per step.
# Region turnaround (consumer -> sem -> refill -> sem -> next consumer)
# is the binding resource; equal regions minimize the worst chain.
REG_W = [1024, 1024, 1024, 1024]
REG_OFF = [0, 1024, 2048, 3072]
N_STEPS = 32              # 32 x 1024 = 32768 cols

# Per-step consumer kind.  Constraint: consecutive same-engine steps must
# not be exactly 4 apart (same region -> serial refill stall).  Default:
# "AADADDAD" x4 with step 28 flipped to A -> 17 A steps (53.1%), matching
# the ACT/DVE rate balance.  Env-overridable.
import os as _os
_KPAT_DEF = "AADADDAD" * 3 + "AADAADAD"
KPAT = _os.environ.get("KERNEL_PATS") or _KPAT_DEF
assert len(KPAT) == N_STEPS and set(KPAT) <= {"A", "D"}

N_WARMUP = 14             # dummy PE matmuls to lift the HAM clock gate
OUT2_FLUSH = 4            # flush out2 staging every this many D steps

# Static maps shared by _build (device) and _combine (host)
_STEPS = []               # (s, tile, kind, width, c0)
ACC_MAP = []              # acc col -> jb tile
OUT2_MAP = []             # (tile, out2 col off, width) per D step
_curs = [0] * NT
_o2 = 0
for _s in range(N_STEPS):
    _t = _s % NT
    _w = REG_W[_s % len(REG_W)]
    _kind = KPAT[_s]
    _STEPS.append((_s, _t, _kind, _w, _curs[_t]))
    _curs[_t] += _w
    if _kind == "A":
        ACC_MAP.append(_t)
    else:
        OUT2_MAP.append((_t, _o2, _w // 2))
        _o2 += _w // 2
assert _curs == [QC] * NT
N_ACC = len(ACC_MAP)
OUT2_W = _o2

_STATE = {}


def _build():
    import concourse.tile as tile
    from concourse import bacc, mybir

    f32 = mybir.dt.float32
    f32r = mybir.dt.float32r
    bf16 = mybir.dt.bfloat16
    i16 = mybir.dt.int16
    Exp = mybir.ActivationFunctionType.Exp
    mul, add = mybir.AluOpType.mult, mybir.AluOpType.add

    nc = bacc.Bacc("TRN2", target_bir_lowering=False, debug=False,
                   num_devices=NCORES)

    v_d = nc.dram_tensor("V2T", (128, JB), f32r, kind="ExternalInput")
    q_d = nc.dram_tensor("queueT", (128, QC), f32r, kind="ExternalInput")
    acc_d = nc.dram_tensor("acc", (128, N_ACC), f32, kind="ExternalOutput")
    out2_d = nc.dram_tensor("out2", (128, OUT2_W), bf16,
                            kind="ExternalOutput")

    with tile.TileContext(nc) as tc:
        with (
            tc.tile_pool(name="const", bufs=1) as const_pool,
            tc.tile_pool(name="sb", bufs=1) as sb_pool,
            tc.tile_pool(name="stag", bufs=3) as stag_pool,
            tc.tile_pool(name="ring", bufs=1, space="PSUM") as ring_pool,
        ):
            ring = ring_pool.tile([128, 4096], f32)

            # PE clock warmup into region 2 (first really used at step 2,
            # by which point the warmup has long retired in PE FIFO order)
            wsrc = const_pool.tile([128, 128], bf16, tag="wsrc")
            nc.vector.memset(wsrc[:], 0.0)
            for i in range(N_WARMUP):
                nc.tensor.matmul(ring[:, 3072:3200], lhsT=wsrc[:],
                                 rhs=wsrc[:], start=True, stop=True)

            # pre-load the ACT Exp table while DMAs are in flight
            twarm = const_pool.tile([128, 8], f32, tag="twarm")
            nc.vector.memset(twarm[:], 0.0)
            nc.scalar.activation(twarm[:], twarm[:], Exp, scale=1.0)

            # host-transposed inputs: V2T [f, jb], qT [f, q] (f32r bytes)
            vt = sb_pool.tile([128, JB], f32r)
            nc.sync.dma_start(vt[:], v_d.ap())
            qt = sb_pool.tile([128, QC], f32r)
            c = 0
            for w in DMA_CHUNKS:
                nc.sync.dma_start(qt[:, c:c + w], q_d.ap()[:, c:c + w])
                c += w

            acc = sb_pool.tile([128, N_ACC], f32)
            pre_all = sb_pool.tile([128, OUT2_W], bf16)

            ai = 0
            o2 = 0
            o2_flushed = 0
            d_since_flush = 0

            def flush_out2(upto):
                nonlocal o2_flushed
                if upto > o2_flushed:
                    nc.sync.dma_start(out2_d.ap()[:, o2_flushed:upto],
                                      pre_all[:, o2_flushed:upto])
                    o2_flushed = upto

            for (s, t, kind, w, c0) in _STEPS:
                r = s % len(REG_W)
                win = ring[:, REG_OFF[r]:REG_OFF[r] + w]
                lhsT = vt[:, t * 128:(t + 1) * 128]
                for p in range(w // 512):
                    nc.tensor.matmul(
                        win[:, p * 512:(p + 1) * 512], lhsT=lhsT,
                        rhs=qt[:, c0 + p * 512:c0 + (p + 1) * 512],
                        start=True, stop=True)
                if kind == "A":
                    nc.scalar.activation(win[:], win[:], Exp, scale=INV_T,
                                         accum_out=acc[:, ai:ai + 1])
                    ai += 1
                else:
                    h = w // 2
                    stag = stag_pool.tile([128, w], i16, tag="stag")
                    nc.vector.tensor_scalar(stag[:], win[:], BT_A, BT_B,
                                            mul, add)
                    nc.gpsimd.tensor_tensor(
                        pre_all[:, o2:o2 + h], stag[:, :h].bitcast(bf16),
                        stag[:, h:].bitcast(bf16), add)
                    o2 += h
                    d_since_flush += 1
                    if d_since_flush >= OUT2_FLUSH:
                        flush_out2(o2)
                        d_since_flush = 0

            flush_out2(OUT2_W)
            nc.sync.dma_start(acc_d.ap(), acc[:])

    nc.compile()
    return nc


def _run(in_maps, trace=False, **kwargs):
    from concourse.bass_utils import run_bass_kernel_spmd
    if "nc" not in _STATE:
        _STATE["nc"] = _build()
    return run_bass_kernel_spmd(_STATE["nc"], in_maps, list(range(NCORES)),
                                trace=trace, **kwargs)


def _make_in_maps(V, L, queue):
    V2T = np.ascontiguousarray(
        np.asarray(V, dtype=np.float32).reshape(JB, F).T)
    qn = np.asarray(queue, dtype=np.float32).reshape(NCORES, QC, F)
    return [{"V2T": V2T,
             "queueT": np.ascontiguousarray(qn[i].T)}
            for i in range(NCORES)]


def _combine(V, L, outs):
    """outs: list of ((128, N_ACC) f32, (128, OUT2_W) bf16) per core."""
    qsum = np.zeros((NT, 128), dtype=np.float64)
    for o, o2 in outs:
        o = o.astype(np.float64)
        for c, t in enumerate(ACC_MAP):
            qsum[t] += o[:, c]
        o2 = o2.astype(np.float64)
        for t, off, h in OUT2_MAP:
            qsum[t] += o2[:, off:off + h].sum(axis=1)
    qsum = qsum.reshape(JB)                                  # jb = t*128 + p
    pos_s = INV_T * np.einsum(
        "jbf,bf->jb",
        np.asarray(V, dtype=np.float64),
        np.asarray(L, dtype=np.float64)).reshape(JB)
    batch_sum = np.exp(pos_s).reshape(J, B).sum(axis=1)      # sum_i exp(pos)
    denom = np.log(np.repeat(batch_sum, B) + qsum)
    loss = -(pos_s.sum() - denom.sum()) / B
    return np.array(loss, dtype=np.float32)


def kernel(V, L, queue):
    res = _run(_make_in_maps(V, L, queue))
    return _combine(V, L, [(res.results[i]["acc"], res.results[i]["out2"])
                           for i in range(NCORES)])
